# revision 1
# baseline (speedup 1.0000x reference)
"""Trainium2 Bass kernel for 3-layer GNN message passing with per-edge
multi-head attention over node history, distributed over 8 NeuronCores.

Sharding: nodes partitioned across cores by id (2500/core); edges sharded by
TARGET node and col-sorted into 128-edge tiles grouped into 128-target
superblocks. Per layer, per-node projection tables (k/v/q rows) are computed
node-sharded on device, assembled on host between launches, and gathered
per-edge via bulk indirect DMA. Segment-sum is a one-hot matmul accumulating
in PSUM per superblock. 4 launches: proj, layer1, layer2, layer3+head.
"""

import sys
import types

import numpy as np

sys.path.insert(0, "/opt/trn_rl_repo")

# ---------------------------------------------------------------- fixups
_HOOK = [None]


def _install_fixups():
    if "antenv.axon_hooks" not in sys.modules:
        mod = types.ModuleType("antenv.axon_hooks")
        mod.set_axon_ntff_profile_hook = lambda h: _HOOK.__setitem__(0, h)
        mod.get_axon_ntff_profile_hook = lambda: _HOOK[0]
        sys.modules["antenv.axon_hooks"] = mod
        try:
            from trn_agent_boot.trn_boot import _ntff_profile_via_ctypes

            _HOOK[0] = _ntff_profile_via_ctypes("/opt/axon/libaxon_pjrt.so")
        except Exception:
            pass

    import concourse.tile as tile
    from concourse.vector_clock import ScopedClock
    import bass_rust

    if getattr(tile.TileContext, "_drain_split_installed", False):
        return

    def _drain_and_barrier(self, tick_clock, wait_clock):
        nc = self.nc
        drain_inst = nc.sync.drain()
        wait_clock.add_sem_waits(
            drain_inst.ins, ScopedClock({None: tick_clock.global_clock})
        )
        si = drain_inst.ins.sync_info
        waits = list(si.on_wait or []) if si is not None else []
        if len(waits) > 1:
            si.on_wait = waits[:1]
            for i in range(1, len(waits)):
                d2 = nc.sync.drain()
                d2.ins.sync_info = bass_rust.SyncInfo(
                    on_wait=waits[i : i + 1], on_update=[]
                )
        nc.all_engine_barrier()
        assert self.sems is not None
        popped = nc._tile_sem_poison_stack.pop()
        assert popped is self._sem_poison
        nc.clear_and_free_semaphores(list(self.sems.allocated().values()))
        nc.all_engine_barrier()

    tile.TileContext._drain_and_barrier = _drain_and_barrier
    tile.TileContext._drain_split_installed = True


# ---------------------------------------------------------------- constants
N = 20000
E = 320000
IN_C = 256
HID = 64
OUT_C = 64
HEADS = 8
DH = 8
NCORES = 8
NPC = N // NCORES  # 2500
SBT = 128  # targets per superblock
NSB = (NPC + SBT - 1) // SBT  # 20
G = 4  # tiles per compute group
SPAN = 16  # tiles per gather DMA

_CACHE = {}


# ---------------------------------------------------------------- host prep
def _preprocess(edge_index):
    row = np.asarray(edge_index[0], dtype=np.int64)
    col = np.asarray(edge_index[1], dtype=np.int64)
    loop = np.arange(N, dtype=np.int64)
    row_all = np.concatenate([row, loop])
    col_all = np.concatenate([col, loop])
    deg = np.bincount(col_all, minlength=N).astype(np.float32)
    dinv = deg**-0.5
    norm = (dinv[row_all] * dinv[col_all]).astype(np.float32)

    per_core = []
    tps = np.zeros(NSB, dtype=np.int64)
    for c in range(NCORES):
        m = (col_all >= c * NPC) & (col_all < (c + 1) * NPC)
        r = row_all[m]
        co = col_all[m] - c * NPC
        nm = norm[m]
        order = np.argsort(co, kind="stable")
        r, co, nm = r[order], co[order], nm[order]
        counts = np.bincount(co // SBT, minlength=NSB)
        per_core.append((r, co, nm, counts))
        tps = np.maximum(tps, (counts + 127) // 128)
    tps = ((tps + G - 1) // G) * G
    tt = int(tps.sum())
    nspan = (tt + SPAN - 1) // SPAN
    tt_pad = nspan * SPAN

    metas = []
    for c in range(NCORES):
        r, co, nm, counts = per_core[c]
        eidx = np.zeros(tt_pad * 128, dtype=np.int32)
        cidx = np.zeros(tt_pad * 128, dtype=np.int32)
        slot = np.zeros(tt_pad * 128, dtype=np.float32)
        nrm = np.zeros(tt_pad * 128, dtype=np.float32)
        ptr = 0
        tile0 = 0
        for k in range(NSB):
            cnt = int(counts[k])
            base = tile0 * 128
            sl = slice(ptr, ptr + cnt)
            eidx[base : base + cnt] = r[sl]
            cidx[base : base + cnt] = co[sl] + c * NPC
            slot[base : base + cnt] = (co[sl] - k * SBT).astype(np.float32)
            nrm[base : base + cnt] = nm[sl]
            ptr += cnt
            tile0 += int(tps[k])
        metas.append(
            dict(
                eidx=np.ascontiguousarray(eidx.reshape(tt_pad, 128).T),
                cidx=np.ascontiguousarray(cidx.reshape(tt_pad, 128).T),
                slot=np.ascontiguousarray(slot.reshape(tt_pad, 128).T),
                nrm=np.ascontiguousarray(nrm.reshape(tt_pad, 128).T),
            )
        )
    return metas, tps, tt, tt_pad


_WS_CTR = [0]


def _split_multi_waits(nc, maxw=1):
    """This container's walrus rejects instructions with more than one sync
    wait; hoist excess waits onto NoOps inserted before the instruction."""
    from concourse import mybir

    for f in nc.m.functions:
        for bb in f.blocks:
            insts = list(bb.instructions)
            out = []
            changed = False
            for inst in insts:
                si = inst.sync_info
                waits = list(si.on_wait) if (si is not None and si.on_wait) else []
                if len(waits) > maxw:
                    excess = waits[: len(waits) - maxw]
                    for j in range(0, len(excess), maxw):
                        _WS_CTR[0] += 1
                        out.append(
                            mybir.InstNoOp(
                                name=f"waitsplit_{_WS_CTR[0]}",
                                engine=inst.engine,
                                sync_info=mybir.SyncInfo(
                                    on_wait=excess[j : j + maxw], on_update=[]
                                ),
                                bass_nofuse=True,
                            )
                        )
                    si.on_wait = waits[len(waits) - maxw :]
                    changed = True
                out.append(inst)
            if changed:
                bb.instructions = out


# ---------------------------------------------------------------- bass helpers
def _mk_nc():
    import concourse.bass as bass

    return bass.Bass(num_devices=NCORES, debug=False, target_bir_lowering=False)


def _const_tiles(nc, pool):
    from concourse import mybir
    from concourse.masks import make_identity

    iota_i = pool.tile([128, 128], mybir.dt.int32)
    nc.gpsimd.iota(iota_i[:], pattern=[[1, 128]], base=0, channel_multiplier=0)
    iota_f = pool.tile([128, 128], mybir.dt.float32)
    nc.vector.tensor_copy(iota_f[:], iota_i[:])
    ident = pool.tile([128, 128], mybir.dt.float32)
    make_identity(nc, ident[:])
    return iota_f, ident


def _load_w(nc, pool, dram_ap, p, f, tag):
    from concourse import mybir

    t = pool.tile([p, f], mybir.dt.float32, tag=tag)
    nc.sync.dma_start(t[:], dram_ap[:])
    return t


def _proj_to_sbuf(nc, psum_pool, dst, pairs, bias, func, nchunk=500):
    """dst = func(sum_i lhsT_i.T @ rhs_i + bias), streamed over node chunks.

    pairs: list of (lhsT_tile_ap, rhs_fn(j0, w) -> AP).
    """
    from concourse import mybir

    n = dst.shape[1]
    for j0 in range(0, n, nchunk):
        w = min(nchunk, n - j0)
        ps = psum_pool.tile([64, nchunk], mybir.dt.float32, tag="proj")
        for i, (lt, rs) in enumerate(pairs):
            nc.tensor.matmul(
                out=ps[:, :w],
                lhsT=lt,
                rhs=rs(j0, w),
                start=(i == 0),
                stop=(i == len(pairs) - 1),
            )
        nc.scalar.activation(dst[:, j0 : j0 + w], ps[:, :w], func, bias=bias)


def _edge_phase(nc, tc, ctx, meta_aps, table, th, tps, tt_pad, iota_f, out_cb, iden_t=None, pseg_bufs=2):
    """Edge pipeline for history length th (1/2/3).

    Table row layout: [k_0..k_{th-1} | v_0..v_{th-1} | q] * 64 (th>1), or
    [v] (th==1). Calls out_cb(k, psum [128,64]) per superblock (pre-relu).
    """
    from concourse import mybir
    import concourse.bass as bass

    f32 = mybir.dt.float32
    kvw = (2 * th if th > 1 else 1) * 64

    meta_pool = ctx.enter_context(tc.tile_pool(name="meta", bufs=1))
    eidx_t = meta_pool.tile([128, tt_pad], mybir.dt.int32)
    nrm_t = meta_pool.tile([128, tt_pad], f32)
    slot_t = meta_pool.tile([128, tt_pad], f32)
    nc.sync.dma_start(eidx_t[:], meta_aps["eidx"][:])
    nc.sync.dma_start(nrm_t[:], meta_aps["nrm"][:])
    nc.sync.dma_start(slot_t[:], meta_aps["slot"][:])


    gat_pool = ctx.enter_context(tc.tile_pool(name="gat", bufs=2))
    q_pool = ctx.enter_context(tc.tile_pool(name="qg", bufs=2))
    tmp_pool = ctx.enter_context(tc.tile_pool(name="etmp", bufs=2))
    sco_pool = ctx.enter_context(tc.tile_pool(name="esco", bufs=2))
    t_pool = ctx.enter_context(tc.tile_pool(name="tmat", bufs=2))
    psum_out = ctx.enter_context(tc.tile_pool(name="pseg", bufs=pseg_bufs, space="PSUM"))
    if th >= 2:
        tt_psum = ctx.enter_context(tc.tile_pool(name="ttp", bufs=2, space="PSUM"))
        qe_psum = ctx.enter_context(tc.tile_pool(name="qep", bufs=2, space="PSUM"))
        qsb_pool = ctx.enter_context(tc.tile_pool(name="qsb", bufs=2))
        tts_pool = ctx.enter_context(tc.tile_pool(name="tts", bufs=2))

    def issue_span(s):
        # one indirect DMA per 128-edge tile (offset [128,1] is the only
        # pattern the walrus unroll honors); SPAN tiles batched per buffer
        if th == 1:
            kv = gat_pool.tile([128, SPAN, 64], f32, tag="kv")
        else:
            kv = gat_pool.tile([128, SPAN, 2 * th, 8, 8], f32, tag="kv")
        for u in range(SPAN):
            nc.gpsimd.indirect_dma_start(
                out=kv[:, u] if th == 1 else kv[:, u].rearrange("p t a b -> p (t a b)"),
                out_offset=None,
                in_=table[:],
                in_offset=bass.IndirectOffsetOnAxis(
                    ap=eidx_t[:, s * SPAN + u : s * SPAN + u + 1], axis=0
                ),
            )
        return (kv, None)

    spans = {}
    sb_of_tile = []
    for k in range(NSB):
        sb_of_tile += [k] * int(tps[k])
    tt = len(sb_of_tile)

    AT = mybir.AluOpType
    ps = None
    done = 0
    for t0 in range(0, tt, G):
        k = sb_of_tile[t0]
        s = t0 // SPAN
        o = t0 - s * SPAN
        if s not in spans:
            spans[s] = issue_span(s)
        kv, qe = spans[s]
        nrm = nrm_t[:, t0 : t0 + G]

        if th >= 2 and done == 0:
            qsb = qsb_pool.tile([128, HID], f32, tag="qsb")
            j0q = k * SBT
            wq_ = min(SBT, NPC - j0q)
            nc.sync.dma_start(qsb[:wq_], meta_aps["qslice"][j0q : j0q + wq_, :])

        tmat = t_pool.tile([128, G, 128], f32, tag="tmat")
        nc.vector.tensor_tensor(
            out=tmat[:],
            in0=slot_t[:, t0 : t0 + G, None].to_broadcast([128, G, 128]),
            in1=iota_f[:, None, :].to_broadcast([128, G, 128]),
            op=AT.is_equal,
        )

        if th == 1:
            ve3 = kv[:, o : o + G, :]  # [128,G,64]
            msg = tmp_pool.tile([128, G, 64], f32, tag="msg")
            nc.vector.tensor_tensor(
                out=msg[:], in0=ve3,
                in1=nrm[:, :, None].to_broadcast([128, G, 64]), op=AT.mult,
            )
        else:
            ke = kv[:, o : o + G, 0:th]  # [128,G,th,8,8]
            if True:
                ttp = tt_psum.tile([128, G, 128], f32, tag="ttp")
                for gi in range(G):
                    nc.tensor.transpose(
                        out=ttp[:, gi], in_=tmat[:, gi], identity=iden_t[:]
                    )
                tts = tts_pool.tile([128, G, 128], f32, tag="tts")
                nc.scalar.copy(tts[:], ttp[:])
                qep = qe_psum.tile([128, G, 8, 8], f32, tag="qep")
                for gi in range(G):
                    nc.tensor.matmul(
                        out=qep[:, gi], lhsT=tts[:, gi], rhs=qsb[:],
                        start=True, stop=True,
                    )
                q4 = qep[:]
            dmul = tmp_pool.tile([128, G, th, 8, 8], f32, tag="dmul")
            nc.vector.tensor_tensor(
                out=dmul[:], in0=ke,
                in1=qep[:, :, None].to_broadcast([128, G, th, 8, 8]),
                op=AT.mult,
            )
            sc = sco_pool.tile([128, G, th, 8], f32, tag="sc")
            nc.vector.tensor_reduce(
                out=sc[:], in_=dmul[:], axis=mybir.AxisListType.X, op=AT.add
            )
            if th == 2:
                z = sco_pool.tile([128, G, 8], f32, tag="z")
                nc.vector.tensor_tensor(
                    out=z[:], in0=sc[:, :, 0], in1=sc[:, :, 1], op=AT.subtract
                )
                a0 = sco_pool.tile([128, G, 8], f32, tag="a0")
                nc.scalar.activation(
                    a0[:], z[:], mybir.ActivationFunctionType.Sigmoid
                )
                an0 = sco_pool.tile([128, G, 8], f32, tag="an0")
                nc.vector.tensor_tensor(
                    out=an0[:], in0=a0[:],
                    in1=nrm[:, :, None].to_broadcast([128, G, 8]), op=AT.mult,
                )
                an1 = sco_pool.tile([128, G, 8], f32, tag="an1")
                nc.vector.tensor_tensor(
                    out=an1[:],
                    in0=nrm[:, :, None].to_broadcast([128, G, 8]),
                    in1=an0[:], op=AT.subtract,
                )
                aw = [an0, an1]
                wv_ = tmp_pool.tile([128, G, 2, 8, 8], f32, tag="wvp")
                for ti in range(2):
                    nc.vector.tensor_tensor(
                        out=wv_[:, :, ti],
                        in0=kv[:, o : o + G, th + ti],
                        in1=aw[ti][:, :, :, None].to_broadcast([128, G, 8, 8]),
                        op=AT.mult,
                    )
                msg = tmp_pool.tile([128, G, 8, 8], f32, tag="msg2")
                nc.vector.tensor_tensor(
                    out=msg[:], in0=wv_[:, :, 0], in1=wv_[:, :, 1], op=AT.add
                )
            else:
                mx = sco_pool.tile([128, G, 8], f32, tag="mx")
                nc.vector.tensor_tensor(
                    out=mx[:], in0=sc[:, :, 0], in1=sc[:, :, 1], op=AT.max
                )
                nc.vector.tensor_tensor(
                    out=mx[:], in0=mx[:], in1=sc[:, :, 2], op=AT.max
                )
                zz = sco_pool.tile([128, G, th, 8], f32, tag="zz")
                nc.vector.tensor_tensor(
                    out=zz[:], in0=sc[:],
                    in1=mx[:, :, None].to_broadcast([128, G, th, 8]),
                    op=AT.subtract,
                )
                ee = sco_pool.tile([128, G, th, 8], f32, tag="ee")
                nc.scalar.activation(
                    ee[:], zz[:], mybir.ActivationFunctionType.Exp
                )
                dd = sco_pool.tile([128, G, 8], f32, tag="dd")
                nc.vector.tensor_tensor(
                    out=dd[:], in0=ee[:, :, 0], in1=ee[:, :, 1], op=AT.add
                )
                nc.vector.tensor_tensor(
                    out=dd[:], in0=dd[:], in1=ee[:, :, 2], op=AT.add
                )
                rr = sco_pool.tile([128, G, 8], f32, tag="rr")
                nc.vector.reciprocal(rr[:], dd[:])
                rn = sco_pool.tile([128, G, 8], f32, tag="rn")
                nc.vector.tensor_tensor(
                    out=rn[:], in0=rr[:],
                    in1=nrm[:, :, None].to_broadcast([128, G, 8]), op=AT.mult,
                )
                aa = sco_pool.tile([128, G, th, 8], f32, tag="aa")
                nc.vector.tensor_tensor(
                    out=aa[:], in0=ee[:],
                    in1=rn[:, :, None].to_broadcast([128, G, th, 8]), op=AT.mult,
                )
                wv_ = tmp_pool.tile([128, G, th, 8, 8], f32, tag="wvp")
                nc.vector.tensor_tensor(
                    out=wv_[:],
                    in0=kv[:, o : o + G, th : 2 * th],
                    in1=aa[:, :, :, :, None].to_broadcast([128, G, th, 8, 8]),
                    op=AT.mult,
                )
                msg = tmp_pool.tile([128, G, 8, 8], f32, tag="msg2")
                nc.vector.tensor_tensor(
                    out=msg[:], in0=wv_[:, :, 0], in1=wv_[:, :, 1], op=AT.add
                )
                nc.vector.tensor_tensor(
                    out=msg[:], in0=msg[:], in1=wv_[:, :, 2], op=AT.add
                )

        if done == 0:
            ps = psum_out.tile([128, 64], f32, tag="ps")
        for gi in range(G):
            nc.tensor.matmul(
                out=ps[:],
                lhsT=tmat[:, gi],
                rhs=msg[:, gi],
                start=(done + gi == 0),
                stop=(done + gi == int(tps[k]) - 1),
            )
        done += G
        if done == int(tps[k]):
            out_cb(k, ps)
            done = 0


def _meta_dram(nc):
    from concourse import mybir

    f32 = mybir.dt.float32
    i32 = mybir.dt.int32
    return {
        "eidx": nc.dram_tensor("eidx", [128, _TTPAD[0]], i32, kind="ExternalInput").ap(),
        "cidx": nc.dram_tensor("cidx", [128, _TTPAD[0]], i32, kind="ExternalInput").ap(),
        "slot": nc.dram_tensor("slot", [128, _TTPAD[0]], f32, kind="ExternalInput").ap(),
        "nrm": nc.dram_tensor("nrm", [128, _TTPAD[0]], f32, kind="ExternalInput").ap(),
    }


_TTPAD = [None]  # set before building


# ---------------------------------------------------------------- launches
def _build_launch_A():
    import concourse.tile as tile
    from concourse import mybir
    from contextlib import ExitStack

    f32 = mybir.dt.float32
    nc = _mk_nc()
    xT = nc.dram_tensor("xT", [IN_C, NPC], f32, kind="ExternalInput").ap()
    w1 = nc.dram_tensor("w1", [IN_C, HID], f32, kind="ExternalInput").ap()
    b1 = nc.dram_tensor("b1", [HID, 1], f32, kind="ExternalInput").ap()
    wv0 = nc.dram_tensor("wv0", [HID, HID], f32, kind="ExternalInput").ap()
    bv0 = nc.dram_tensor("bv0", [HID, 1], f32, kind="ExternalInput").ap()
    hT_out = nc.dram_tensor("hT_out", [HID, NPC], f32, kind="ExternalOutput").ap()
    v1_rows = nc.dram_tensor("v1_rows", [NPC, HID], f32, kind="ExternalOutput").ap()

    with tile.TileContext(nc) as tc, ExitStack() as ctx:
        cpool = ctx.enter_context(tc.tile_pool(name="const", bufs=1))
        from concourse.masks import make_identity

        ident = cpool.tile([128, 128], f32)
        make_identity(nc, ident[:])

        wpool = ctx.enter_context(tc.tile_pool(name="w", bufs=1))
        xpool = ctx.enter_context(tc.tile_pool(name="x", bufs=2))
        hpool = ctx.enter_context(tc.tile_pool(name="h", bufs=1))
        act_pool = ctx.enter_context(tc.tile_pool(name="act", bufs=2))
        psum_pool = ctx.enter_context(tc.tile_pool(name="ps", bufs=2, space="PSUM"))
        tp_pool = ctx.enter_context(tc.tile_pool(name="tp", bufs=2, space="PSUM"))

        w1a = _load_w(nc, wpool, w1[0:128, :], 128, HID, "w1a")
        w1b = _load_w(nc, wpool, w1[128:256, :], 128, HID, "w1b")
        b1t = _load_w(nc, wpool, b1, HID, 1, "b1t")
        wv0t = _load_w(nc, wpool, wv0, HID, HID, "wv0t")
        bv0t = _load_w(nc, wpool, bv0, HID, 1, "bv0t")

        hT = hpool.tile([HID, NPC], f32)
        v1T = hpool.tile([HID, NPC], f32)

        NCH = 500
        for j0 in range(0, NPC, NCH):
            w = min(NCH, NPC - j0)
            xa = xpool.tile([128, NCH], f32, tag="xa")
            xb = xpool.tile([128, NCH], f32, tag="xb")
            nc.sync.dma_start(xa[:, :w], xT[0:128, j0 : j0 + w])
            nc.sync.dma_start(xb[:, :w], xT[128:256, j0 : j0 + w])
            ps = psum_pool.tile([HID, NCH], f32, tag="p1")
            nc.tensor.matmul(out=ps[:, :w], lhsT=w1a[:], rhs=xa[:, :w], start=True, stop=False)
            nc.tensor.matmul(out=ps[:, :w], lhsT=w1b[:], rhs=xb[:, :w], start=False, stop=True)
            nc.scalar.activation(
                hT[:, j0 : j0 + w], ps[:, :w],
                mybir.ActivationFunctionType.Relu, bias=b1t[:],
            )
            ps2 = psum_pool.tile([HID, NCH], f32, tag="p2")
            nc.tensor.matmul(out=ps2[:, :w], lhsT=wv0t[:], rhs=hT[:, j0 : j0 + w], start=True, stop=True)
            nc.scalar.activation(
                v1T[:, j0 : j0 + w], ps2[:, :w],
                mybir.ActivationFunctionType.Identity, bias=bv0t[:],
            )
        nc.sync.dma_start(hT_out[:], hT[:])

        for j0 in range(0, NPC, 128):
            w = min(128, NPC - j0)
            ps = tp_pool.tile([128, HID], f32, tag="tp")
            nc.tensor.transpose(
                out=ps[:w, :], in_=v1T[:, j0 : j0 + w], identity=ident[:HID, :HID]
            )
            sb = act_pool.tile([128, HID], f32, tag="ro")
            nc.scalar.copy(sb[:w], ps[:w])
            nc.sync.dma_start(v1_rows[j0 : j0 + w, :], sb[:w])
    _split_multi_waits(nc)
    return nc


def _build_launch_mid(layer, tps, tt, tt_pad):
    import concourse.tile as tile
    from concourse import mybir
    from contextlib import ExitStack

    f32 = mybir.dt.float32
    th = layer
    nl = layer + 1
    nc = _mk_nc()
    roww = 64 if th == 1 else (2 * th + 1) * 64
    next_roww = (2 * nl + 1) * 64
    table = nc.dram_tensor("table", [N, roww], f32, kind="ExternalInput").ap()
    meta_aps = _meta_dram(nc)
    if layer == 2:
        meta_aps["qslice"] = nc.dram_tensor(
            "qslice", [NPC, HID], f32, kind="ExternalInput"
        ).ap()
    hists_d = [nc.dram_tensor("histT0", [HID, NPC], f32, kind="ExternalInput").ap()]
    if layer == 2:
        hists_d.append(
            nc.dram_tensor("histT1", [HID, NPC], f32, kind="ExternalInput").ap()
        )
    wk = nc.dram_tensor("wk", [HID, HID], f32, kind="ExternalInput").ap()
    wv = nc.dram_tensor("wv", [HID, HID], f32, kind="ExternalInput").ap()
    wq = nc.dram_tensor("wq", [HID, HID], f32, kind="ExternalInput").ap()
    bk = nc.dram_tensor("bk", [HID, 1], f32, kind="ExternalInput").ap()
    bv = nc.dram_tensor("bv", [HID, 1], f32, kind="ExternalInput").ap()
    bq = nc.dram_tensor("bq", [HID, 1], f32, kind="ExternalInput").ap()
    outT_d = nc.dram_tensor("outT", [HID, NPC], f32, kind="ExternalOutput").ap()
    rows_d = nc.dram_tensor("rows", [NPC, next_roww], f32, kind="ExternalOutput").ap()

    with tile.TileContext(nc) as tc, ExitStack() as ctx:
        cpool = ctx.enter_context(tc.tile_pool(name="const", bufs=1))
        iota_f, ident = _const_tiles(nc, cpool)
        wpool = ctx.enter_context(tc.tile_pool(name="w", bufs=1))
        hpool = ctx.enter_context(tc.tile_pool(name="h", bufs=1))
        act_pool = ctx.enter_context(tc.tile_pool(name="act", bufs=2))
        pb = 1 if layer == 2 else 2
        psum_m = ctx.enter_context(tc.tile_pool(name="psm", bufs=pb, space="PSUM"))
        tp_pool = ctx.enter_context(tc.tile_pool(name="tp", bufs=pb, space="PSUM"))

        wkt = _load_w(nc, wpool, wk, HID, HID, "wkt")
        wvt = _load_w(nc, wpool, wv, HID, HID, "wvt")
        wqt = _load_w(nc, wpool, wq, HID, HID, "wqt")
        bkt = _load_w(nc, wpool, bk, HID, 1, "bkt")
        bvt = _load_w(nc, wpool, bv, HID, 1, "bvt")
        bqt = _load_w(nc, wpool, bq, HID, 1, "bqt")

        histT = []
        for i, hd in enumerate(hists_d):
            ht = hpool.tile([HID, NPC], f32, tag=f"hist{i}")
            nc.sync.dma_start(ht[:], hd[:])
            histT.append(ht)
        outT = hpool.tile([HID, NPC], f32, tag="outT")

        def out_cb(k, ps):
            j0 = k * SBT
            w = min(SBT, NPC - j0)
            sb = act_pool.tile([128, HID], f32, tag="oc")
            nc.scalar.activation(sb[:w], ps[:w], mybir.ActivationFunctionType.Relu)
            tp = tp_pool.tile([HID, 128], f32, tag="ot")
            nc.tensor.transpose(out=tp[:, :w], in_=sb[:w], identity=ident[:w, :w])
            nc.scalar.copy(outT[:, j0 : j0 + w], tp[:, :w])

        _edge_phase(
            nc, tc, ctx, meta_aps, table, th, tps, tt_pad, iota_f, out_cb,
            iden_t=ident, pseg_bufs=(1 if layer == 2 else 2),
        )

        nc.sync.dma_start(outT_d[:], outT[:])

        allh = histT + [outT]
        colTs = []
        Ident = mybir.ActivationFunctionType.Identity
        for i, hsrc in enumerate(allh):
            kt = hpool.tile([HID, NPC], f32, tag=f"kT{i}")
            _proj_to_sbuf(
                nc, psum_m, kt[:],
                [(wkt[:], lambda j0, w, hs=hsrc: hs[:, j0 : j0 + w])], bkt[:], Ident,
            )
            colTs.append(kt)
        for i, hsrc in enumerate(allh):
            vt = hpool.tile([HID, NPC], f32, tag=f"vT{i}")
            _proj_to_sbuf(
                nc, psum_m, vt[:],
                [(wvt[:], lambda j0, w, hs=hsrc: hs[:, j0 : j0 + w])], bvt[:], Ident,
            )
            colTs.append(vt)
        qt = hpool.tile([HID, NPC], f32, tag="qT")
        _proj_to_sbuf(
            nc, psum_m, qt[:],
            [(wqt[:], lambda j0, w: outT[:, j0 : j0 + w])], bqt[:], Ident,
        )
        colTs.append(qt)

        tp2 = ctx.enter_context(tc.tile_pool(name="tp2", bufs=pb, space="PSUM"))
        for j0 in range(0, NPC, 128):
            w = min(128, NPC - j0)
            ps = tp2.tile([128, next_roww], f32, tag="rw")
            for i, ct in enumerate(colTs):
                nc.tensor.transpose(
                    out=ps[:w, i * 64 : (i + 1) * 64],
                    in_=ct[:, j0 : j0 + w],
                    identity=ident[:HID, :HID],
                )
            sb = act_pool.tile([128, next_roww], f32, tag="rwsb")
            nc.scalar.copy(sb[:w], ps[:w])
            nc.sync.dma_start(rows_d[j0 : j0 + w, :], sb[:w])
    _split_multi_waits(nc)
    return nc


def _build_launch_D(tps, tt, tt_pad):
    import concourse.tile as tile
    from concourse import mybir
    from contextlib import ExitStack

    f32 = mybir.dt.float32
    th = 3
    nc = _mk_nc()
    roww = (2 * th + 1) * 64
    table = nc.dram_tensor("table", [N, roww], f32, kind="ExternalInput").ap()
    meta_aps = _meta_dram(nc)
    meta_aps["qslice"] = nc.dram_tensor(
        "qslice", [NPC, HID], f32, kind="ExternalInput"
    ).ap()
    w2 = nc.dram_tensor("w2", [HID, OUT_C], f32, kind="ExternalInput").ap()
    b2bc = nc.dram_tensor("b2bc", [128, OUT_C], f32, kind="ExternalInput").ap()
    y_d = nc.dram_tensor("y", [NPC, OUT_C], f32, kind="ExternalOutput").ap()

    with tile.TileContext(nc) as tc, ExitStack() as ctx:
        cpool = ctx.enter_context(tc.tile_pool(name="const", bufs=1))
        iota_f, ident = _const_tiles(nc, cpool)
        wpool = ctx.enter_context(tc.tile_pool(name="w", bufs=1))
        act_pool = ctx.enter_context(tc.tile_pool(name="act", bufs=2))
        tp_pool = ctx.enter_context(tc.tile_pool(name="tp", bufs=1, space="PSUM"))
        lg_pool = ctx.enter_context(tc.tile_pool(name="lg", bufs=1, space="PSUM"))
        sm_pool = ctx.enter_context(tc.tile_pool(name="sm", bufs=2))

        w2t = _load_w(nc, wpool, w2, HID, OUT_C, "w2t")
        b2t = _load_w(nc, wpool, b2bc, 128, OUT_C, "b2t")
        AT = mybir.AluOpType

        def out_cb(k, ps):
            j0 = k * SBT
            w = min(SBT, NPC - j0)
            o3 = act_pool.tile([128, HID], f32, tag="o3")
            nc.scalar.activation(o3[:w], ps[:w], mybir.ActivationFunctionType.Relu)
            tp = tp_pool.tile([HID, 128], f32, tag="o3t")
            nc.tensor.transpose(out=tp[:, :w], in_=o3[:w], identity=ident[:w, :w])
            o3T = act_pool.tile([HID, 128], f32, tag="o3T")
            nc.scalar.copy(o3T[:, :w], tp[:, :w])
            lg = lg_pool.tile([128, OUT_C], f32, tag="lg")
            nc.tensor.matmul(
                out=lg[:w], lhsT=o3T[:, :w], rhs=w2t[:], start=True, stop=True
            )
            logits = sm_pool.tile([128, OUT_C], f32, tag="logits")
            nc.vector.tensor_tensor(out=logits[:w], in0=lg[:w], in1=b2t[:w], op=AT.add)
            lmax = sm_pool.tile([128, 1], f32, tag="lmax")
            nc.vector.tensor_reduce(
                out=lmax[:w], in_=logits[:w], axis=mybir.AxisListType.X, op=AT.max
            )
            zz = sm_pool.tile([128, OUT_C], f32, tag="zzs")
            nc.vector.tensor_tensor(
                out=zz[:w], in0=logits[:w],
                in1=lmax[:w].to_broadcast([w, OUT_C]), op=AT.subtract,
            )
            eb = sm_pool.tile([128, OUT_C], f32, tag="eb")
            esum = sm_pool.tile([128, 1], f32, tag="esum")
            nc.scalar.activation(
                eb[:w], zz[:w], mybir.ActivationFunctionType.Exp, accum_out=esum[:w]
            )
            lse = sm_pool.tile([128, 1], f32, tag="lse")
            nc.scalar.activation(lse[:w], esum[:w], mybir.ActivationFunctionType.Ln)
            yy = sm_pool.tile([128, OUT_C], f32, tag="yy")
            nc.vector.tensor_tensor(
                out=yy[:w], in0=zz[:w],
                in1=lse[:w].to_broadcast([w, OUT_C]), op=AT.subtract,
            )
            nc.sync.dma_start(y_d[j0 : j0 + w, :], yy[:w])

        _edge_phase(
            nc, tc, ctx, meta_aps, table, th, tps, tt_pad, iota_f, out_cb,
            iden_t=ident,
        )
    _split_multi_waits(nc)
    return nc


# ---------------------------------------------------------------- driver
def kernel(x, edge_index, lin1_w, lin1_b, wq, bq, wk, bk, wv, bv, lin2_w, lin2_b):
    _install_fixups()
    from concourse.bass_utils import run_bass_kernel_spmd

    x = np.asarray(x, dtype=np.float32)
    lin1_w = np.asarray(lin1_w, np.float32)
    lin1_b = np.asarray(lin1_b, np.float32)
    wq = np.asarray(wq, np.float32)
    bq = np.asarray(bq, np.float32)
    wk = np.asarray(wk, np.float32)
    bk = np.asarray(bk, np.float32)
    wv = np.asarray(wv, np.float32)
    bv = np.asarray(bv, np.float32)
    lin2_w = np.asarray(lin2_w, np.float32)
    lin2_b = np.asarray(lin2_b, np.float32)

    metas, tps, tt, tt_pad = _preprocess(np.asarray(edge_index))

    key = ("progs", tuple(tps.tolist()), tt_pad)
    if key not in _CACHE:
        _TTPAD[0] = tt_pad
        _CACHE[key] = (
            _build_launch_A(),
            _build_launch_mid(1, tps, tt, tt_pad),
            _build_launch_mid(2, tps, tt, tt_pad),
            _build_launch_D(tps, tt, tt_pad),
        )
    ncA, ncB, ncC, ncD = _CACHE[key]

    isd = np.float32(1.0 / np.sqrt(DH))
    xT = np.ascontiguousarray(x.T)
    cores = list(range(NCORES))

    in_maps = [
        dict(
            xT=np.ascontiguousarray(xT[:, c * NPC : (c + 1) * NPC]),
            w1=lin1_w,
            b1=lin1_b[:, None],
            wv0=wv[0],
            bv0=bv[0][:, None],
        )
        for c in cores
    ]
    resA = run_bass_kernel_spmd(ncA, in_maps, cores)
    hT = [resA.results[c]["hT_out"] for c in cores]
    v1_table = np.ascontiguousarray(
        np.concatenate([resA.results[c]["v1_rows"] for c in cores], axis=0)
    )

    in_maps = [
        dict(
            table=v1_table,
            eidx=metas[c]["eidx"], cidx=metas[c]["cidx"],
            slot=metas[c]["slot"], nrm=metas[c]["nrm"],
            histT0=hT[c],
            wk=wk[1], wv=wv[1], wq=np.ascontiguousarray(wq[1] * isd),
            bk=bk[1][:, None], bv=bv[1][:, None],
            bq=np.ascontiguousarray((bq[1] * isd))[:, None],
        )
        for c in cores
    ]
    resB = run_bass_kernel_spmd(ncB, in_maps, cores)
    out1T = [resB.results[c]["outT"] for c in cores]
    kvq2_table = np.ascontiguousarray(
        np.concatenate([resB.results[c]["rows"] for c in cores], axis=0)
    )

    in_maps = [
        dict(
            table=kvq2_table,
            eidx=metas[c]["eidx"], cidx=metas[c]["cidx"],
            slot=metas[c]["slot"], nrm=metas[c]["nrm"],
            histT0=hT[c], histT1=out1T[c],
            qslice=np.ascontiguousarray(
                kvq2_table[c * NPC : (c + 1) * NPC, 4 * 64 :]
            ),
            wk=wk[2], wv=wv[2], wq=np.ascontiguousarray(wq[2] * isd),
            bk=bk[2][:, None], bv=bv[2][:, None],
            bq=np.ascontiguousarray((bq[2] * isd))[:, None],
        )
        for c in cores
    ]
    resC = run_bass_kernel_spmd(ncC, in_maps, cores)
    kvq3_table = np.ascontiguousarray(
        np.concatenate([resC.results[c]["rows"] for c in cores], axis=0)
    )

    b2bc = np.ascontiguousarray(np.broadcast_to(lin2_b[None, :], (128, OUT_C)))
    in_maps = [
        dict(
            table=kvq3_table,
            eidx=metas[c]["eidx"], cidx=metas[c]["cidx"],
            slot=metas[c]["slot"], nrm=metas[c]["nrm"],
            qslice=np.ascontiguousarray(
                kvq3_table[c * NPC : (c + 1) * NPC, 6 * 64 :]
            ),
            w2=lin2_w, b2bc=b2bc,
        )
        for c in cores
    ]
    resD = run_bass_kernel_spmd(ncD, in_maps, cores)
    return np.concatenate([resD.results[c]["y"] for c in cores], axis=0)

